# revision 20
# baseline (speedup 1.0000x reference)
"""Trainium2 Bass kernel for BudgetAttentionTwo.

Module: keys = x@Wk.T+bk, values = x@Wv.T+bv (split into 8 heads of 64),
S = K K^T per (b, h), out = (softmax(S)/sqrt(E)) @ V, merged back to [B,N,E].

Sharding: 8 cores, each core owns one batch b = core//2 and four heads
hg*4..hg*4+3 (hg = core%2). No cross-device comms. Weights are pre-sliced
and pre-transposed on the host.

v2 design (ACT-bound pipeline):
  - The ACT engine's exp stream (16.7M elems/core @ 1 elem/lane/cyc @1.2GHz
    ~= 110us + per-instr overhead) is the hard bottleneck; everything else is
    scheduled to keep it gapless.
  - Scores as before: S^T block [128 k, 512 q] via block-diagonal rhs pairs
    (even/odd heads share one LDWEIGHTS; fp32r).
  - P = exp(S - 88) written bf16 (ample range: P <= e^31; quantization
    ~0.4% el-wise, averages out in attV) into pts2 [128, 2 j, 8192].
  - attV in "P-as-weights" orientation for ALL heads: per (head, q-chunk of
    128): accumulate 16 k-chunk matmuls with lhsT = P chunk [128 k, 128 q]
    (bf16 -> compiler FWL, loads on the separate weight port at ~53ns) and
    rhs = [V|ones] [128 k, 65] bf16 moving (N=65, ~30ns). Out = [128 q, 65]
    fp32 psum, 4 q-chunks packed per bank. This frees the PE moving port
    (attV moving drops 54.6us -> ~30us), the rowsum rides along as col 64,
    and normalization becomes a per-partition scalar multiply (the old
    broadcast-matmul + 3.3us DVE reciprocal epilogue is gone). Output is
    [q, d] so the host needs no transpose.
  - PSUM start-flag rule: within a packed acc bank only the very FIRST
    matmul uses start=True (bank-wide has_written clear); all others rely on
    per-element has_written (unset -> overwrite, set -> accumulate).
  - Software pipeline: attV of stage s-1 + its normalize/DMA are emitted as
    filler slices between the score groups of stage s, so the PE always has
    work while ACT drains exp, and ACT never waits on PE. Stage 0's filler
    is the V projection; stage 1's is K-projection(pair 1) then attV(0).
  - Input DMAs split across the sync and gpsimd queues so the first exp
    lands ~15us in (was 65us).
"""
import numpy as np

import concourse.bacc as bacc
import concourse.mybir as mybir
import concourse.tile as tile
from concourse.bass_utils import run_bass_kernel_spmd

F32 = mybir.dt.float32
F32R = mybir.dt.float32r
BF16 = mybir.dt.bfloat16
F16 = mybir.dt.float16
EXP = mybir.ActivationFunctionType.Exp

B, N, E, H = 4, 2048, 512, 8
D = E // H            # 64
NCORES = 8
HPC = 4               # heads per core
CSHIFT = 88.0         # exp(S - CSHIFT)
QW = 512              # q-range width per stage
NQR = N // QW         # 4
KC = N // 128         # 16 k-chunks
GRP = 3               # score chunks per psum tile (3 banks)
DP1 = D + 1           # 65: V plus trailing ones column

_last_results = None  # stashed BassKernelResults for test.py introspection


def _register_const(nc, val):
    """Extra pre-TileContext f32 [128,1] constant (dep-free, like Bass's
    built-in consts) so activation(bias=val) needs no semaphore wait."""
    t = nc.alloc_sbuf_tensor(f"const-float32-{val}", [128, 1], F32)
    nc.gpsimd.memset(t.ap(), val)
    nc.const_aps.aps[(F32, float(val))] = t.ap()
    nc.all_engine_barrier()


def build_program():
    nc = bacc.Bacc()
    _register_const(nc, -CSHIFT)

    xt = nc.dram_tensor("xt", [E, N], F16, kind="ExternalInput")
    wkt = nc.dram_tensor("wkt", [E, 2 * 128], F16, kind="ExternalInput")
    wvt = nc.dram_tensor("wvt", [E, 2 * 128], F16, kind="ExternalInput")
    bk2 = nc.dram_tensor("bk2", [2, 128, 1], F32, kind="ExternalInput")
    bvb = nc.dram_tensor("bvb", [128, HPC * D], F32, kind="ExternalInput")
    vinit = nc.dram_tensor("vinit", [128, HPC * DP1], BF16, kind="ExternalInput")
    zrow = nc.dram_tensor("zrow", [1, N], F16, kind="ExternalInput")
    out_t = nc.dram_tensor("out_t", [HPC, N, D], F32, kind="ExternalOutput")

    with nc.allow_low_precision(reason="fp32r/bf16 rounding for PE speed is intentional"), \
         tile.TileContext(nc) as tc:
        with (
            tc.tile_pool(name="persist", bufs=1) as per,
            tc.tile_pool(name="work", bufs=2) as work,
            tc.tile_pool(name="ptp", bufs=1) as ptp,
            tc.tile_pool(name="mps", bufs=1, space="PSUM") as mps,
        ):
            # ---- persistent SBUF ----
            kt2 = [per.tile([128, N], F16, name=f"kt2_{p}") for p in range(2)]
            # block-diagonal rhs copies: bd[0][p] = [KT_even; 0],
            # bd[1][p] = [0; KT_odd] so the score matmul pair shares one
            # LDWEIGHTS of kt2 while contracting over 128 partitions.
            bd = [[per.tile([128, N], F16, name=f"bd_{j}_{p}")
                   for p in range(2)] for j in range(2)]
            vs = [per.tile([128, HPC * DP1], BF16, name=f"vs_{t}")
                  for t in range(KC)]
            bvb_sb = per.tile([128, HPC * D], F32)
            bk_sb = [per.tile([128, 1], F32, name=f"bk_{p}") for p in range(2)]
            xt_sb = [per.tile([128, N], F16, name=f"xt_{c}") for c in range(4)]
            wkt_sb = [per.tile([128, 2 * 128], F16, name=f"wkt_{c}")
                      for c in range(4)]
            wvt_sb = [per.tile([128, 2 * 128], F16, name=f"wvt_{c}")
                      for c in range(4)]

            # ---- input DMAs, ordered so K-projection inputs land first:
            # sync:   wkt0, xt0, wkt1-3, xt2, bk   (proj-critical)
            # gpsimd: xt1, xt3, bd-zeros, wvt, bvb, vinit (V side later)
            nc.sync.dma_start(out=wkt_sb[0], in_=wkt[0:128, :])
            nc.sync.dma_start(out=xt_sb[0], in_=xt[0:128, :])
            nc.gpsimd.dma_start(out=xt_sb[1], in_=xt[128:256, :])
            for c in range(1, 4):
                nc.sync.dma_start(out=wkt_sb[c],
                                  in_=wkt[128 * c:128 * (c + 1), :])
            nc.sync.dma_start(out=xt_sb[2], in_=xt[256:384, :])
            nc.gpsimd.dma_start(out=xt_sb[3], in_=xt[384:512, :])
            for p in range(2):
                nc.sync.dma_start(out=bk_sb[p], in_=bk2[p])
            # zero the off-head halves of the block-diagonal tiles; p=0's
            # are needed by the first score matmuls (~18us) -> sync queue,
            # p=1's are not needed until stage 4 -> gpsimd, last
            zb = zrow[:].partition_broadcast(64)
            nc.sync.dma_start(out=bd[0][0][64:128, :], in_=zb)
            nc.sync.dma_start(out=bd[1][0][0:64, :], in_=zb)
            for c in range(4):
                nc.gpsimd.dma_start(out=wvt_sb[c],
                                    in_=wvt[128 * c:128 * (c + 1), :])
            nc.gpsimd.dma_start(out=bvb_sb, in_=bvb[:])
            nc.gpsimd.dma_start(out=bd[0][1][64:128, :], in_=zb)
            nc.gpsimd.dma_start(out=bd[1][1][0:64, :], in_=zb)

            # ---- PE warm-up: the HAM clock gate holds the PE at 1.2 GHz
            # until ~3.4us of sustained activity; the input-DMA window
            # (~8-14us) would otherwise leave the projections and first
            # score groups running at half clock. Chew on the first wkt
            # chunk (read-only, junk psum output) to span that window.
            warm = mps.tile([128, 256], F32, tag="acc", bufs=2, name="warm")
            for _ in range(20):
                nc.tensor.matmul(
                    warm[:], wkt_sb[0][:, 0:128], wkt_sb[0][:],
                    start=True, stop=True,
                )

            def proj_kt2(p, qr):
                # KT2[p][:, q-range] = (Wk_pair @ x^T + bk_pair), fp32r
                acc = mps.tile([128, QW], F32, tag="acc", bufs=2,
                               name=f"kacc_{p}_{qr}")
                for c in range(4):
                    nc.tensor.matmul(
                        acc[:],
                        wkt_sb[c][:, 128 * p:128 * (p + 1)],
                        xt_sb[c][:, QW * qr:QW * (qr + 1)],
                        start=(c == 0), stop=(c == 3),
                    )
                qs = slice(QW * qr, QW * (qr + 1))
                nc.vector.tensor_scalar_add(kt2[p][:, qs], acc[:], bk_sb[p][:])
                nc.vector.tensor_scalar_add(bd[0][p][0:64, qs],
                                            acc[0:64, :], bk_sb[p][0:64])
                nc.vector.tensor_scalar_add(bd[1][p][64:128, qs],
                                            acc[64:128, :], bk_sb[p][64:128])

            def proj_v(t):
                # V tile [128 n, 4 heads * 65] + bias + ones col, bf16,
                # pre-scaled by 1/sqrt(E) on the host (wvt, bvb).
                acc = mps.tile([128, QW], F32, tag="acc", bufs=2,
                               name=f"vacc_{t}")
                for c in range(4):
                    nc.tensor.matmul(
                        acc[:, :HPC * D],
                        xt_sb[c][:, 128 * t:128 * (t + 1)],
                        wvt_sb[c][:],
                        start=(c == 0), stop=(c == 3),
                    )
                vst = vs[t].rearrange("p (h y) -> p h y", h=HPC)
                if t == 0:
                    nc.gpsimd.dma_start(out=vs[t], in_=vinit[:])
                else:
                    v0 = vs[0].rearrange("p (h y) -> p h y", h=HPC)
                    nc.vector.tensor_copy(vst[:, :, D], v0[:, :, D])
                nc.vector.tensor_tensor(
                    out=vst[:, :, 0:D],
                    in0=acc[:, :HPC * D].rearrange("p (h d) -> p h d", h=HPC),
                    in1=bvb_sb.rearrange("p (h d) -> p h d", h=HPC),
                    op=mybir.AluOpType.add,
                )

            def make_attv(si, p, qr, pts2, last=False):
                """attV for stage (p, qr): 7 emission slices (6 matmul
                groups chasing the exp groups + finalize)."""
                acc = [mps.tile([128, HPC * DP1], F32, tag="acc", bufs=2,
                                name=f"acc_{si}_{j}") for j in range(2)]
                q0 = QW * qr
                slices = []
                for g in range(0, KC, GRP):
                    kcs = list(range(g, min(g + GRP, KC)))

                    def emit(kcs=kcs):
                        for kc in kcs:
                            vsl = vs[kc].rearrange("p (h y) -> p h y", h=HPC)
                            for j in range(2):
                                for qc in range(4):
                                    nc.tensor.matmul(
                                        acc[j][:, DP1 * qc:DP1 * (qc + 1)],
                                        pts2[:, j,
                                             QW * kc + 128 * qc:
                                             QW * kc + 128 * (qc + 1)],
                                        vsl[:, 2 * p + j, :],
                                        start=(kc == 0 and qc == 0),
                                        stop=(kc == KC - 1 and qc == 3),
                                        skip_group_check=True,
                                    )
                    slices.append(emit)

                def finalize():
                    for j in range(2):
                        hl = 2 * p + j
                        accv = acc[j].rearrange("p (qc y) -> p qc y", y=DP1)
                        rec = work.tile([128, 4], F32, tag="rec", bufs=2,
                                        name=f"rec_{si}_{j}")
                        nc.vector.reciprocal(rec[:], accv[:, :, D])
                        fin = work.tile([128, HPC * D], F32, tag="fin",
                                        bufs=2, name=f"fin_{si}_{j}")
                        for qc in range(4):
                            nc.vector.tensor_scalar_mul(
                                fin[:, D * qc:D * (qc + 1)],
                                accv[:, qc, 0:D], rec[:, qc:qc + 1])
                        # one batched DMA per head: dram rows q0+qi*128+p
                        dst = out_t[hl, q0:q0 + QW, :].rearrange(
                            "(qi p) d -> p qi d", p=128)
                        eng = nc.sync if (last or j == 0) else nc.gpsimd
                        eng.dma_start(
                            out=dst,
                            in_=fin.rearrange("p (qi d) -> p qi d", qi=4))
                slices.append(finalize)
                return slices

            def stage(si, p, qr, fillers):
                q0 = QW * qr
                pts2 = ptp.tile([128, 2, KC * QW], BF16, tag="pt", bufs=3,
                                name=f"pts_{si}")
                fi = 0
                for g in range(0, KC, GRP):
                    w = min(GRP, KC - g)
                    sc = [mps.tile([128, GRP * QW], F32, tag="sc", bufs=2,
                                   name=f"sc_{si}_{g}_{j}") for j in range(2)]
                    for i in range(w):
                        kc = g + i
                        for j in range(2):
                            nc.tensor.matmul(
                                sc[j][:, QW * i:QW * (i + 1)],
                                kt2[p][:, 128 * kc:128 * (kc + 1)],
                                bd[j][p][:, q0:q0 + QW],
                                start=True, stop=True,
                            )
                    if fi < len(fillers):
                        fillers[fi]()
                        fi += 1
                    for j in range(2):
                        nc.scalar.activation(
                            pts2[:, j, QW * g:QW * (g + w)],
                            sc[j][:, :QW * w],
                            EXP, bias=-CSHIFT, scale=1.0,
                        )
                for f in fillers[fi:]:
                    f()
                return pts2

            # ---- emission ----
            proj_kt2(0, 0)

            stages = [(p, qr) for p in range(2) for qr in range(NQR)]
            pending = None   # attV slices of the previous stage
            for si, (p, qr) in enumerate(stages):
                if si == 0:
                    # rest of the K-projection(pair 0) feeds score groups
                    # just-in-time; V projection fills the remaining slack
                    fillers = [lambda q2=q2: proj_kt2(0, q2)
                               for q2 in range(1, NQR)]
                    splits = (3, 8, 12, 16)
                    for a, b in zip((0,) + splits, splits):
                        ts = list(range(a, b))
                        fillers.append(lambda ts=ts: [proj_v(t) for t in ts])
                elif si == 1:
                    fillers = [lambda qr2=qr2: proj_kt2(1, qr2)
                               for qr2 in range(NQR)]
                    fillers += pending
                else:
                    fillers = pending
                pts2 = stage(si, p, qr, fillers)
                pending = make_attv(si, p, qr, pts2,
                                    last=(si == len(stages) - 1))
            for f in pending:
                f()

    nc.finalize()
    return nc


_program = None


def kernel(x, Wk, bk, Wv, bv):
    global _program, _last_results
    import ml_dtypes
    x = np.asarray(x, dtype=np.float32)
    Wk = np.asarray(Wk, dtype=np.float32)
    bk = np.asarray(bk, dtype=np.float32)
    Wv = np.asarray(Wv, dtype=np.float32)
    bv = np.asarray(bv, dtype=np.float32)

    if _program is None:
        _program = build_program()

    sq = np.float32(1.0 / np.sqrt(E))
    vi = np.zeros((128, HPC * DP1), dtype=ml_dtypes.bfloat16)
    vi[:, D::DP1] = 1.0
    in_maps = []
    for c in range(NCORES):
        b, hg = c // 2, c % 2
        cols = slice(hg * HPC * D, (hg + 1) * HPC * D)
        in_maps.append({
            "xt": np.ascontiguousarray(x[b].T).astype(np.float16),   # [E, N]
            "wkt": np.ascontiguousarray(Wk[cols, :].T).astype(np.float16),
            "wvt": (np.ascontiguousarray(Wv[cols, :].T) * sq).astype(np.float16),
            "bk2": np.ascontiguousarray(bk[cols].reshape(2, 128, 1)),
            "bvb": np.ascontiguousarray(
                np.broadcast_to(bv[cols] * sq, (128, HPC * D))),
            "vinit": vi,
            "zrow": np.zeros((1, N), dtype=np.float16),
        })

    import os
    trace = bool(int(os.environ.get("KERNEL_PROFILE", "0")))
    res = run_bass_kernel_spmd(_program, in_maps, list(range(NCORES)),
                               trace=trace)
    _last_results = res

    out = np.empty((B, N, E), dtype=np.float32)
    for c in range(NCORES):
        b, hg = c // 2, c % 2
        ot = res.results[c]["out_t"]                                 # [4, N, 64]
        for hl in range(HPC):
            out[b, :, hg * HPC * D + hl * D:(hg * HPC * D) + (hl + 1) * D] = \
                ot[hl]
    return out
